# revision 1
# baseline (speedup 1.0000x reference)
"""MoE kernel v3: pair-wise F-split (2 cores per expert pair, F/2 each).

Experts are paired largest-with-smallest; the pair's two cores each hold
the F-half of BOTH experts (128 KB/partition, same as v1) and process all
tokens of both experts on their half. Slot capacities are global
(CA = largest expert count, CB = largest count among the 4 "small" slot
experts), so the program is SPMD; per-core data decides which experts a
core serves. Partial outputs (bf16) from the two cores of a pair are
summed on host, then combined/scattered as in v1.

Per-core PE work: (CA + CB) columns x 256 cycles — ~4% less than v1's
2*max_count x 256, with essentially v1's DMA volume.

DRAM layouts per core (FL = F/2 = 2048, FLO = FL/128 = 16):
  x   [n_tiles, 128, KO, CT] bf16  slot-A tiles then slot-B tiles
  w1  [2, 4, 128, KO, 512]   bf16  w1[s,q,p,ko,ff] = w1_{e_s}[ko*128+p, h*FL+q*512+ff]
  w2  [2, 2, 128, 8, D]      bf16  w2[s,b,p,fi,d]  = w2_{e_s}[h*FL+(b*8+fi)*128+p, d]
  b1  [128, 2*FLO]           f32   b1[p, s*FLO+fq] = b1_{e_s}[h*FL+fq*128+p]
  y   [n_tiles, 128, KO, CT] bf16  partial (gelu(x@w1l+b1l) @ w2l)^T
(h = the core's half index within its pair.)
"""

import numpy as np
import ml_dtypes

N_CORES = 8
D = 1024
F = 4096
E = 8
KO = D // 128
FL = F // 2          # 2048 local F columns per core
FLO = FL // 128      # 16 local f-chunks
CT = 512

BF16 = ml_dtypes.bfloat16

_NC_CACHE: dict[tuple, object] = {}
LAST_RESULTS = None


def _cap_tiles(C):
    tiles = []
    off = 0
    while C - off >= CT:
        tiles.append((off, CT))
        off += CT
    if off < C:
        tiles.append((off, C - off))
    return tiles


def _build(CA, CB):
    import concourse.mybir as mybir
    from concourse import bacc
    from concourse.tile import TileContext

    fp32 = mybir.dt.float32
    bf16 = mybir.dt.bfloat16

    spec = [(0, off, tw) for off, tw in _cap_tiles(CA)] + [
        (1, off, tw) for off, tw in _cap_tiles(CB)
    ]
    n_tiles = len(spec)

    nc = bacc.Bacc(
        "TRN2", target_bir_lowering=False, debug=False, num_devices=N_CORES
    )
    x = nc.dram_tensor("x", [n_tiles, 128, KO, CT], bf16, kind="ExternalInput")
    w1 = nc.dram_tensor("w1", [2, 4, 128, KO, 512], bf16, kind="ExternalInput")
    w2 = nc.dram_tensor("w2", [2, 2, 128, 8, D], bf16, kind="ExternalInput")
    b1 = nc.dram_tensor("b1", [128, 2 * FLO], fp32, kind="ExternalInput")
    y = nc.dram_tensor("y", [n_tiles, 128, KO, CT], bf16, kind="ExternalOutput")

    with TileContext(nc) as tc:
        with (
            tc.tile_pool(name="wpool", bufs=1) as wpool,
            tc.tile_pool(name="xpool", bufs=4) as xpool,
            tc.tile_pool(name="hpool", bufs=2) as hpool,
            tc.tile_pool(name="ypool", bufs=4) as ypool,
            tc.tile_pool(name="ph", bufs=4, space="PSUM") as phpool,
            tc.tile_pool(name="py", bufs=4, space="PSUM") as pypool,
        ):
            w1_sb = wpool.tile([128, 2, 4, KO, 512], bf16)
            w2_sb = wpool.tile([128, 2, FLO, D], bf16)
            b1_sb = wpool.tile([128, 2 * FLO], fp32)

            x_first = xpool.tile([128, KO, CT], bf16, tag="x_sb")
            nc.sync.dma_start(x_first[:], x[0])
            # Slot A's w1 quarters first (PE starts after 1 MB), then its
            # w2 (mm2 needs it ~30us in), then slot B's weights.
            for q in range(4):
                nc.sync.dma_start(w1_sb[:, 0, q], w1[0, q])
            nc.sync.dma_start(b1_sb[:], b1[:])
            for b in range(2):
                nc.sync.dma_start(w2_sb[:, 0, b * 8 : (b + 1) * 8, :], w2[0, b])
            for q in range(4):
                nc.sync.dma_start(w1_sb[:, 1, q], w1[1, q])
            for b in range(2):
                nc.sync.dma_start(w2_sb[:, 1, b * 8 : (b + 1) * 8, :], w2[1, b])

            for ti, (s, off, tw) in enumerate(spec):
                if ti == 0:
                    x_sb = x_first
                else:
                    x_sb = xpool.tile([128, KO, CT], bf16, tag="x_sb")
                    nc.sync.dma_start(x_sb[:], x[ti])
                h_sb = hpool.tile([128, FLO, CT], bf16)
                for fo in range(FLO):
                    q, fq = divmod(fo, 4)
                    ph = phpool.tile([128, CT], fp32)
                    for ko in range(KO):
                        nc.tensor.matmul(
                            ph[:, :tw],
                            lhsT=w1_sb[:, s, q, ko, fq * 128 : (fq + 1) * 128],
                            rhs=x_sb[:, ko, :tw],
                            start=(ko == 0),
                            stop=(ko == KO - 1),
                        )
                    nc.scalar.activation(
                        h_sb[:, fo, :tw],
                        ph[:, :tw],
                        mybir.ActivationFunctionType.Gelu,
                        bias=b1_sb[:, s * FLO + fo : s * FLO + fo + 1],
                    )
                for do in range(KO):
                    py = pypool.tile([128, CT], fp32)
                    for fo in range(FLO):
                        nc.tensor.matmul(
                            py[:, :tw],
                            lhsT=w2_sb[:, s, fo, do * 128 : (do + 1) * 128],
                            rhs=h_sb[:, fo, :tw],
                            start=(fo == 0),
                            stop=(fo == FLO - 1),
                        )
                    y_do = ypool.tile([128, CT], bf16, tag="y_do")
                    nc.vector.tensor_copy(y_do[:, :tw], py[:, :tw])
                    # Full-width DMA: contiguous rows (128 descriptors, no
                    # strided slow path); pad columns carry ignored stale
                    # data. Per-do DMAs pipeline under the remaining mm2s,
                    # so the kernel tail only waits on one 128 KB transfer.
                    nc.sync.dma_start(y[ti][:, do, :], y_do[:])

    nc.compile()
    return nc, spec


def kernel(x, gate_w, w1, b1, w2, b2):
    from concourse.bass_utils import run_bass_kernel_spmd

    global LAST_RESULTS

    x = np.asarray(x, dtype=np.float32)
    gate_w = np.asarray(gate_w, dtype=np.float32)
    w1 = np.asarray(w1, dtype=np.float32)
    b1 = np.asarray(b1, dtype=np.float32)
    w2 = np.asarray(w2, dtype=np.float32)
    b2 = np.asarray(b2, dtype=np.float32)

    B, S, Din = x.shape
    assert Din == D and gate_w.shape == (D, E)
    T = B * S
    xf = x.reshape(T, D)

    # ---- Host router + dispatch (as v1) ----
    logits = xf.astype(np.float64) @ gate_w.astype(np.float64)
    idx0 = np.argmax(logits, axis=1)
    rows = np.arange(T)
    v0 = logits[rows, idx0]
    l2 = logits.copy()
    l2[rows, idx0] = -np.inf
    idx1 = np.argmax(l2, axis=1)
    v1_ = l2[rows, idx1]
    e1 = np.exp(v1_ - v0)
    cw0 = 1.0 / (1.0 + e1)
    cw1 = e1 / (1.0 + e1)

    token_ids = []
    combine_w = []
    for e in range(E):
        sel0 = idx0 == e
        sel1 = idx1 == e
        ids = np.nonzero(sel0 | sel1)[0]
        w = np.where(sel0[ids], cw0[ids], cw1[ids])
        token_ids.append(ids)
        combine_w.append(w)

    counts = np.array([len(ids) for ids in token_ids])
    # Pair i-th largest with i-th smallest; slot A = the large expert.
    order = np.argsort(-counts)
    pairs = [(int(order[i]), int(order[E - 1 - i])) for i in range(E // 2)]
    CA = int(max(counts[eA] for eA, _ in pairs))
    CB = int(max(counts[eB] for _, eB in pairs))
    CA += CA & 1
    CB += CB & 1

    if (CA, CB) not in _NC_CACHE:
        _NC_CACHE[(CA, CB)] = _build(CA, CB)
    nc, spec = _NC_CACHE[(CA, CB)]
    n_tiles = len(spec)

    # ---- Per-pair token tiles; per-core weight halves ----
    in_maps = [None] * N_CORES
    pair_x = []
    for pi, (eA, eB) in enumerate(pairs):
        xtiles = np.zeros((n_tiles, 128, KO, CT), dtype=BF16)
        for ti, (s, off, tw) in enumerate(spec):
            e = (eA, eB)[s]
            ids_seg = token_ids[e][off : off + tw]
            w_val = len(ids_seg)
            if w_val == 0:
                continue
            blk = (
                xf[ids_seg].astype(BF16).reshape(w_val, KO, 128).transpose(2, 1, 0)
            )
            xtiles[ti, :, :, :w_val] = blk
        xtiles = np.ascontiguousarray(xtiles)
        pair_x.append(xtiles)
        for h in range(2):
            sl = slice(h * FL, (h + 1) * FL)
            w1c = np.stack(
                [
                    w1[e][:, sl]
                    .reshape(KO, 128, 4, 512)
                    .transpose(2, 1, 0, 3)
                    for e in (eA, eB)
                ]
            ).astype(BF16)  # [2, 4, 128, KO, 512]
            w2c = np.stack(
                [
                    w2[e][sl, :]
                    .reshape(2, 8, 128, D)
                    .transpose(0, 2, 1, 3)
                    for e in (eA, eB)
                ]
            ).astype(BF16)  # [2, 2, 128, 8, D]
            b1c = np.concatenate(
                [b1[e][sl].reshape(FLO, 128).T for e in (eA, eB)], axis=1
            )  # [128, 2*FLO]
            in_maps[2 * pi + h] = {
                "x": xtiles,
                "w1": np.ascontiguousarray(w1c),
                "w2": np.ascontiguousarray(w2c),
                "b1": np.ascontiguousarray(b1c),
            }

    res = run_bass_kernel_spmd(nc, in_maps, core_ids=list(range(N_CORES)))
    LAST_RESULTS = res

    # ---- Host: sum the pair halves, combine, scatter ----
    out = np.zeros((T, D), dtype=np.float32)
    for pi, (eA, eB) in enumerate(pairs):
        ysum = res.results[2 * pi]["y"].astype(np.float32) + res.results[
            2 * pi + 1
        ]["y"].astype(np.float32)
        for ti, (s, off, tw) in enumerate(spec):
            e = (eA, eB)[s]
            ids_seg = token_ids[e][off : off + tw]
            w_val = len(ids_seg)
            if w_val == 0:
                continue
            cw_seg = combine_w[e][off : off + w_val].astype(np.float32)
            yt = ysum[ti, :, :, :w_val].transpose(2, 1, 0).reshape(w_val, D)
            out[ids_seg] += cw_seg[:, None] * (yt + b2[e])

    return out.reshape(B, S, D)



# revision 8
# speedup vs baseline: 1.0085x; 1.0085x over previous
"""MoE kernel v4: F/8 expert-slice scheme (all experts on every core).

Each core holds a 512-wide F-slice of ALL 8 experts (w1 [1024, 512] +
w2 [512, 1024] per expert, 16.8 MB bf16 total) and processes ALL 16384
token-assignment columns; its mm2 output is a partial over its F-slice,
and the 8 partials are summed on host. This is perfectly SPMD with ZERO
load-balance padding (every core does identical work; only the weight
contents differ), unlike the v3 pair scheme which padded to global slot
capacities (CA+CB = 4204 vs ideal 4096 columns).

Tokens are dispatched (host router, top-2) into per-expert column
groups, tiled at <=512 columns per tile with every tile single-expert.
Expert remainders < 256 columns are split with a borrowed full tile so
all tile widths are >= 256 (keeps FWL weight loads hidden under the
matmul stream).

Per tile (tw cols): mm1 = 4 fo x 8 ko matmuls -> gelu(+b1) -> h
[128,4,512] bf16; mm2 = 8 do x 4 fi matmuls -> cast bf16 -> y tile.
Issue order software-pipelines mm1(t+1) before mm2(t) so gelu latency
never stalls the PE. ~20 junk matmuls on a zeroed scratch tile warm the
HAM clock gate during the initial DMA wait.

DRAM layouts per core (FL = F/8 = 512):
  x   [NT, 128, KO, 512] bf16  x[t][p, ko, c] = xf[ids[c], ko*128+p]
  w1  [E, 128, KO, FL]   bf16  w1[e][p, ko, f] = w1_e[ko*128+p, cF*FL+f]
  w2  [E, 128, 4, D]     bf16  w2[e][p, fi, d] = w2_e[cF*FL+fi*128+p, d]
  b1  [128, E*4]         f32   b1[p, e*4+fq]   = b1_e[cF*FL+fq*128+p]
  y   [NT, 128, KO, 512] bf16  partial (gelu(x@w1l+b1l) @ w2l)^T
(cF = the core id = which F-slice it owns.)
"""

import numpy as np
import ml_dtypes

N_CORES = 8
D = 1024
F = 4096
E = 8
KO = D // 128
FL = F // N_CORES     # 512 local F columns per core
FQ = FL // 128        # 4 local f-chunks
CT = 512

BF16 = ml_dtypes.bfloat16

_NC_CACHE: dict[tuple, object] = {}
LAST_RESULTS = None


def _expert_tiles(c):
    """Split c columns into tile widths <=512, all >=256 when possible."""
    if c == 0:
        return []
    n_full, rem = divmod(c, CT)
    if rem == 0:
        return [CT] * n_full
    if rem >= 256 or n_full == 0:
        return [CT] * n_full + [rem]
    a = (CT + rem) // 2
    a += a & 1
    return [CT] * (n_full - 1) + [a, CT + rem - a]


def _build(spec):
    import concourse.mybir as mybir
    from concourse import bacc
    from concourse.tile import TileContext

    fp32 = mybir.dt.float32
    bf16 = mybir.dt.bfloat16
    Gelu = mybir.ActivationFunctionType.Gelu

    NT = len(spec)
    e_order = []
    for e, _ in spec:
        if e not in e_order:
            e_order.append(e)

    nc = bacc.Bacc(
        "TRN2", target_bir_lowering=False, debug=False, num_devices=N_CORES
    )
    x = nc.dram_tensor("x", [NT, 128, KO, CT], bf16, kind="ExternalInput")
    w1 = nc.dram_tensor("w1", [E, 128, KO, FL], bf16, kind="ExternalInput")
    w2 = nc.dram_tensor("w2", [E, 128, FQ, D], bf16, kind="ExternalInput")
    b1 = nc.dram_tensor("b1", [128, E * FQ], fp32, kind="ExternalInput")
    y = nc.dram_tensor("y", [NT, 128, KO, CT], bf16, kind="ExternalOutput")

    with TileContext(nc) as tc:
        with (
            tc.tile_pool(name="wpool", bufs=1) as wpool,
            tc.tile_pool(name="xpool", bufs=4) as xpool,
            tc.tile_pool(name="hpool", bufs=3) as hpool,
            tc.tile_pool(name="ypool", bufs=3) as ypool,
            tc.tile_pool(name="ph", bufs=4, space="PSUM") as phpool,
            tc.tile_pool(name="py", bufs=4, space="PSUM") as pypool,
        ):
            w1_sb = wpool.tile([128, E, KO, FL], bf16)
            w2_sb = wpool.tile([128, E, FQ, D], bf16)
            b1_sb = wpool.tile([128, E * FQ], fp32)
            scr = wpool.tile([128, CT], bf16)

            nc.vector.memset(scr[:], 0.0)

            # DMA order: critical path first (x0, first expert's w1), then
            # the rest roughly in order of first use.
            x_tiles = [xpool.tile([128, KO, CT], bf16, tag="x_sb", name="x_sb")
                       for _ in range(min(4, NT))]
            nc.sync.dma_start(x_tiles[0][:], x[0])
            nc.sync.dma_start(w1_sb[:, e_order[0]], w1[e_order[0]])
            nc.sync.dma_start(b1_sb[:], b1[:])
            if NT > 1:
                nc.sync.dma_start(x_tiles[1][:], x[1])
            nc.sync.dma_start(w2_sb[:, e_order[0]], w2[e_order[0]])
            for i in range(2, min(4, NT)):
                nc.sync.dma_start(x_tiles[i][:], x[i])
            for e in e_order[1:]:
                nc.sync.dma_start(w1_sb[:, e], w1[e])
                nc.sync.dma_start(w2_sb[:, e], w2[e])

            # HAM warmup: junk matmuls on the zeroed scratch tile keep the
            # PE busy (and the clock gate open) while the first DMAs land.
            ph_w = phpool.tile([128, CT], fp32, tag="ph")
            for _ in range(20):
                nc.tensor.matmul(
                    ph_w[:], lhsT=scr[:, :128], rhs=scr[:],
                    start=True, stop=True,
                )

            def mm1(t):
                e, tw = spec[t]
                x_sb = x_tiles[t]
                h_sb = hpool.tile([128, FQ, CT], bf16, tag="h_sb")
                for fo in range(FQ):
                    ph = phpool.tile([128, CT], fp32, tag="ph")
                    for ko in range(KO):
                        nc.tensor.matmul(
                            ph[:, :tw],
                            lhsT=w1_sb[:, e, ko, fo * 128:(fo + 1) * 128],
                            rhs=x_sb[:, ko, :tw],
                            start=(ko == 0),
                            stop=(ko == KO - 1),
                        )
                    nc.scalar.activation(
                        h_sb[:, fo, :tw],
                        ph[:, :tw],
                        Gelu,
                        bias=b1_sb[:, e * FQ + fo: e * FQ + fo + 1],
                    )
                return h_sb

            def mm2(t, h_sb):
                e, tw = spec[t]
                y_sb = ypool.tile([128, KO, CT], bf16, tag="y_sb")
                for do in range(KO):
                    py = pypool.tile([128, CT], fp32)
                    for fi in range(FQ):
                        nc.tensor.matmul(
                            py[:, :tw],
                            lhsT=w2_sb[:, e, fi, do * 128:(do + 1) * 128],
                            rhs=h_sb[:, fi, :tw],
                            start=(fi == 0),
                            stop=(fi == FQ - 1),
                        )
                    nc.vector.tensor_copy(y_sb[:, do, :tw], py[:, :tw])
                nc.sync.dma_start(y[t], y_sb[:])

            h_prev = mm1(0)
            for t in range(NT):
                if t + 1 < NT:
                    h_next = mm1(t + 1)
                else:
                    h_next = None
                if t + 4 < NT:
                    x_sb = xpool.tile([128, KO, CT], bf16, tag="x_sb")
                    nc.sync.dma_start(x_sb[:], x[t + 4])
                    x_tiles.append(x_sb)
                mm2(t, h_prev)
                h_prev = h_next

    nc.compile()
    return nc


def kernel(x, gate_w, w1, b1, w2, b2):
    from concourse.bass_utils import run_bass_kernel_spmd

    global LAST_RESULTS

    x = np.asarray(x, dtype=np.float32)
    gate_w = np.asarray(gate_w, dtype=np.float32)
    w1 = np.asarray(w1, dtype=np.float32)
    b1 = np.asarray(b1, dtype=np.float32)
    w2 = np.asarray(w2, dtype=np.float32)
    b2 = np.asarray(b2, dtype=np.float32)

    B, S, Din = x.shape
    assert Din == D and gate_w.shape == (D, E)
    T = B * S
    xf = x.reshape(T, D)

    # ---- Host router + dispatch ----
    logits = xf.astype(np.float64) @ gate_w.astype(np.float64)
    idx0 = np.argmax(logits, axis=1)
    rows = np.arange(T)
    v0 = logits[rows, idx0]
    l2 = logits.copy()
    l2[rows, idx0] = -np.inf
    idx1 = np.argmax(l2, axis=1)
    v1_ = l2[rows, idx1]
    e1 = np.exp(v1_ - v0)
    cw0 = 1.0 / (1.0 + e1)
    cw1 = e1 / (1.0 + e1)

    token_ids = []
    combine_w = []
    for e in range(E):
        sel0 = idx0 == e
        sel1 = idx1 == e
        ids = np.nonzero(sel0 | sel1)[0]
        w = np.where(sel0[ids], cw0[ids], cw1[ids])
        token_ids.append(ids)
        combine_w.append(w)

    # ---- Tile spec: per-expert tiles over the global column stream ----
    spec = []            # (expert, tile_width)
    tile_seg = []        # (expert, start offset into token_ids[e]) per tile
    for e in range(E):
        off = 0
        for tw in _expert_tiles(len(token_ids[e])):
            spec.append((e, tw))
            tile_seg.append((e, off))
            off += tw
    NT = len(spec)

    key = tuple(spec)
    if key not in _NC_CACHE:
        _NC_CACHE[key] = _build(spec)
    nc = _NC_CACHE[key]

    # ---- Shared x dispatch (same array for every core) ----
    xtiles = np.zeros((NT, 128, KO, CT), dtype=BF16)
    for ti, ((e, tw), (_, off)) in enumerate(zip(spec, tile_seg)):
        ids_seg = token_ids[e][off: off + tw]
        blk = xf[ids_seg].astype(BF16).reshape(tw, KO, 128).transpose(2, 1, 0)
        xtiles[ti, :, :, :tw] = blk
    xtiles = np.ascontiguousarray(xtiles)

    # ---- Per-core weight F-slices ----
    in_maps = []
    for c in range(N_CORES):
        sl = slice(c * FL, (c + 1) * FL)
        w1c = np.stack(
            [w1[e][:, sl].reshape(KO, 128, FL).transpose(1, 0, 2)
             for e in range(E)]
        ).astype(BF16)                       # [E, 128, KO, FL]
        w2c = np.stack(
            [w2[e][sl, :].reshape(FQ, 128, D).transpose(1, 0, 2)
             for e in range(E)]
        ).astype(BF16)                       # [E, 128, FQ, D]
        b1c = np.concatenate(
            [b1[e][sl].reshape(FQ, 128).T for e in range(E)], axis=1
        )                                    # [128, E*FQ]
        in_maps.append({
            "x": xtiles,
            "w1": np.ascontiguousarray(w1c),
            "w2": np.ascontiguousarray(w2c),
            "b1": np.ascontiguousarray(b1c),
        })

    res = run_bass_kernel_spmd(nc, in_maps, core_ids=list(range(N_CORES)))
    LAST_RESULTS = res

    # ---- Host: sum F-slice partials, combine, scatter ----
    ysum = np.zeros((NT, 128, KO, CT), dtype=np.float32)
    for c in range(N_CORES):
        ysum += res.results[c]["y"].astype(np.float32)

    out = np.zeros((T, D), dtype=np.float32)
    for ti, ((e, tw), (_, off)) in enumerate(zip(spec, tile_seg)):
        ids_seg = token_ids[e][off: off + tw]
        cw_seg = combine_w[e][off: off + tw].astype(np.float32)
        yt = ysum[ti, :, :, :tw].transpose(2, 1, 0).reshape(tw, D)
        out[ids_seg] += cw_seg[:, None] * (yt + b2[e])

    return out.reshape(B, S, D)
